# revision 1
# baseline (speedup 1.0000x reference)
"""MoE FFN (8 experts, top-2, GLU) on 8 Trainium2 NeuronCores.

Strategy
--------
Phase 1 (on-device, data-parallel over tokens): each core computes router
logits for its 512-token shard, softmax-free top-2 gate weights
c[t, e] = z_e / (z_1 + z_2) for the two largest z (z = exp(logit - max)),
zero elsewhere.  Exactly matches softmax + top-k + L1-normalize.

Host dispatch (data movement only): for each expert, gather the columns of
x^T for its routed tokens into a fixed-capacity buffer.

Phase 2 (on-device, expert-parallel): core e computes the GLU FFN of expert
e over its gathered tokens with full-rate float32r matmuls:
    h = silu(w1t^T xg) * (v1t^T xg)   [F, C]
    y = h^T w2                        [C, H]
    y *= c                            (per-token gate weight)

Host combine (data movement only): out[idx_e] += y_e.

Weights are shipped pre-tiled ([FO,128,HO,128] etc.) so every weight DMA is
a single large contiguous descriptor per partition, and streamed in
half-tiles so the PE never waits on a full weight block.  x chunks, weight
tiles and outputs are spread over the SWDGE/HWDGE queues so the serial DMA
engine feeds the PE in dependency order.

The expert phase computes only the exact active token count CA (rounded to
8 for fp32r free-dim alignment) while buffers stay at capacity C, trimming
dead matmul rows; the h tail [CA:ceil128(CA)] is zeroed so padded psum
columns stay finite and gate to zero.

Measured (seed-0 inputs, 8 cores): relative error 2.77e-4 vs the fp32
reference; timeline-sim exec time ~208 us total (router 13.9 us + expert
~194 us at CA=1072; expert PE-busy ~178 us ~= the fp32r roofline).
"""

import numpy as np

import concourse.bacc as bacc
import concourse.mybir as mybir
import concourse.tile as tile
from concourse.bass_utils import run_bass_kernel_spmd

P = 128
E = 8
H = 1024
F = 2048
T = 4096
NCORES = 8
TSH = T // NCORES  # tokens per core in router phase
HO = H // P  # 8
FO = F // P  # 16
F32 = mybir.dt.float32
F32R = mybir.dt.float32r

_NC_CACHE = {}
_W_CACHE = {}
_USE_SILU_ACT = True


def _token_chunks(C):
    """Split C into free-dim chunks <= 512 (>= 256 when C >= 512)."""
    assert C >= 1
    chunks = []
    t0 = 0
    while t0 < C:
        rem = C - t0
        if rem >= 768:
            tl = 512
        elif rem > 512:
            tl = rem - 256  # leaves a final 256 chunk; both >= 256
        else:
            tl = rem
        chunks.append((t0, tl))
        t0 += tl
    return chunks


def _build_router():
    nc = bacc.Bacc("TRN2", target_bir_lowering=False, debug=False,
                   enable_partition_id=False)
    xT = nc.dram_tensor("xT", [H, TSH], F32, kind="ExternalInput")
    rwT = nc.dram_tensor("rwT", [H, E], F32, kind="ExternalInput")
    c_out = nc.dram_tensor("c", [TSH, E], F32, kind="ExternalOutput")
    with tile.TileContext(nc) as tc:
        with tc.tile_pool(name="xp", bufs=1) as xp, \
             tc.tile_pool(name="wp", bufs=1) as wp, \
             tc.tile_pool(name="sp", bufs=4) as sp, \
             tc.tile_pool(name="ps", bufs=4, space="PSUM") as ps:
            rw = wp.tile([P, HO, E], F32)
            nc.sync.dma_start(rw[:], rwT.ap().rearrange("(ho p) e -> p ho e", p=P))
            HH = HO // 2
            xts = []
            for tt in range(TSH // P):
                halves = []
                for hf in range(2):
                    xt = xp.tile([P, HH, P], F32, tag=f"xt{tt}_{hf}",
                                 name=f"xt{tt}_{hf}")
                    nc.sync.dma_start(
                        xt[:],
                        xT.ap()[hf * HH * P:(hf + 1) * HH * P,
                                tt * P:(tt + 1) * P].rearrange(
                            "(ho p) t -> p ho t", p=P))
                    halves.append(xt)
                xts.append(halves)
            pls = [ps.tile([P, E], F32, tag="pl", name=f"pl{tt}")
                   for tt in range(TSH // P)]
            for tt in range(TSH // P):
                for ho in range(HO):
                    nc.tensor.matmul(pls[tt][:],
                                     xts[tt][ho // HH][:, ho % HH, :],
                                     rw[:, ho, :],
                                     start=(ho == 0), stop=(ho == HO - 1))
            for tt in range(TSH // P):
                pl = pls[tt]
                nmax = sp.tile([P, 1], F32, tag="nmax")
                nc.vector.tensor_reduce(nmax[:], pl[:], axis=mybir.AxisListType.X,
                                        op=mybir.AluOpType.max, negate=True)
                z = sp.tile([P, E], F32, tag="z")
                nc.scalar.activation(z[:], pl[:], mybir.ActivationFunctionType.Exp,
                                     bias=nmax[:, 0:1], scale=1.0)
                m8 = sp.tile([P, 8], F32, tag="m8")
                nc.vector.max(m8[:], z[:])
                s2 = sp.tile([P, 1], F32, tag="s2")
                nc.vector.tensor_reduce(s2[:], m8[:, 0:2], axis=mybir.AxisListType.X,
                                        op=mybir.AluOpType.add)
                rec = sp.tile([P, 1], F32, tag="rec")
                nc.vector.reciprocal(rec[:], s2[:])
                cm = sp.tile([P, E], F32, tag="cm")
                nc.vector.tensor_scalar(cm[:], z[:], m8[:, 1:2], None,
                                        op0=mybir.AluOpType.is_ge)
                cg = sp.tile([P, E], F32, tag="cg")
                nc.vector.tensor_mul(cg[:], z[:], cm[:])
                nc.vector.tensor_scalar_mul(cg[:], cg[:], rec[:, 0:1])
                nc.sync.dma_start(c_out.ap()[tt * P:(tt + 1) * P, :], cg[:])
    nc.compile()
    return nc


def _build_expert(C, CA=None):
    if CA is None:
        CA = C
    assert 1 <= CA <= C
    CA = min(C, ((CA + 7) // 8) * 8)  # fp32r free-dim alignment
    TO = (CA + P - 1) // P  # active 128-token blocks
    CH = TO * P             # h width (>= CA, <= C)
    nc = bacc.Bacc("TRN2", target_bir_lowering=False, debug=False,
                   enable_partition_id=False)
    xgT = nc.dram_tensor("xgT", [H, C], F32R, kind="ExternalInput")
    cgt = nc.dram_tensor("cgt", [P, TO], F32, kind="ExternalInput")
    w1t = nc.dram_tensor("w1t", [FO, P, HO, P], F32R, kind="ExternalInput")
    v1t = nc.dram_tensor("v1t", [FO, P, HO, P], F32R, kind="ExternalInput")
    w2t = nc.dram_tensor("w2t", [2, P, FO, H // 2], F32R, kind="ExternalInput")
    y = nc.dram_tensor("y", [C, H], F32, kind="ExternalOutput")
    chunks = _token_chunks(CA)
    with tile.TileContext(nc) as tc:
        with tc.tile_pool(name="xp", bufs=1) as xp, \
             tc.tile_pool(name="hp", bufs=1) as hp, \
             tc.tile_pool(name="wp", bufs=2) as wp, \
             tc.tile_pool(name="w2p", bufs=28) as w2p, \
             tc.tile_pool(name="cp", bufs=1) as cp, \
             tc.tile_pool(name="yp", bufs=6) as yp, \
             tc.tile_pool(name="ps", bufs=3, space="PSUM") as ps, \
             tc.tile_pool(name="psb", bufs=2, space="PSUM") as psb:
            HH = HO // 2

            def load_wv_half(fo, half):
                w1s = wp.tile([P, HH, P], F32R, tag=f"w1s{half}",
                              name=f"w1s{fo}_{half}")
                nc.sync.dma_start(
                    w1s[:], w1t.ap()[fo, :, half * HH:(half + 1) * HH, :])
                v1s = wp.tile([P, HH, P], F32R, tag=f"v1s{half}",
                              name=f"v1s{fo}_{half}")
                nc.sync.dma_start(
                    v1s[:], v1t.ap()[fo, :, half * HH:(half + 1) * HH, :])
                return (w1s, v1s)

            def load_wv(fo):
                return [load_wv_half(fo, 0), load_wv_half(fo, 1)]

            def wsl(halves, mat, ho):
                return halves[ho // HH][mat][:, ho % HH, :]

            xgs = []
            for ho in range(HO):
                xgc = xp.tile([P, CA], F32R, tag=f"xg{ho}", name=f"xg{ho}")
                dma = nc.sync.dma_start if ho == 0 else nc.gpsimd.dma_start
                dma(xgc[:], xgT.ap()[ho * P:(ho + 1) * P, :CA])
                xgs.append(xgc)
            h = hp.tile([P, FO, CH], F32R)
            if CA < CH:
                nc.vector.memset(h[:, :, CA:CH].bitcast(mybir.dt.uint32), 0)
            cgs = cp.tile([P, TO], F32)
            nc.gpsimd.dma_start(cgs[:], cgt.ap())


            def glu_tail(fo, t0, tl, p1, p2):
                hs = h[:, fo, t0:t0 + tl]
                if _USE_SILU_ACT:
                    nc.scalar.activation(hs, p1,
                                         mybir.ActivationFunctionType.Silu)
                    nc.vector.tensor_mul(hs, hs, p2)
                else:
                    # silu(a)*b = a*sigmoid(a)*b (sim fallback)
                    sg = yp.tile([P, 512], F32, tag="sg", name="sg")[:, :tl]
                    nc.scalar.activation(sg, p1,
                                         mybir.ActivationFunctionType.Sigmoid)
                    nc.vector.tensor_mul(hs, p1, sg)
                    nc.vector.tensor_mul(hs, hs, p2)

            if len(chunks) > 3:
                # psum-budget fallback: chunk-serial accumulation
                for fo in range(FO):
                    wv = load_wv(fo)
                    for ci, (t0, tl) in enumerate(chunks):
                        p1 = ps.tile([P, 512], F32, tag="ps1", name="p1")[:, :tl]
                        p2 = ps.tile([P, 512], F32, tag="ps2", name="p2")[:, :tl]
                        for ho in range(HO):
                            st, sp_ = (ho == 0), (ho == HO - 1)
                            nc.tensor.matmul(p1, wsl(wv, 0, ho),
                                             xgs[ho][:, t0:t0 + tl],
                                             start=st, stop=sp_)
                            nc.tensor.matmul(p2, wsl(wv, 1, ho),
                                             xgs[ho][:, t0:t0 + tl],
                                             start=st, stop=sp_)
                        glu_tail(fo, t0, tl, p1, p2)
            else:
                    # Phase A: h = silu(w1t^T @ xg) * (v1t^T @ xg), laid out [f, t].
                # Prologue runs fo=0 (all chunks) plus fo=1's first chunk with
                # ho-outer order so the PE chases the streaming xg chunks.
                wv0 = load_wv(0)
                wv1 = load_wv(1)
                t00, tl0 = chunks[0]
                ps1s = [ps.tile([P, 512], F32, tag="ps1", name=f"ps1_{i}")[:, :tl]
                        for i, (t0, tl) in enumerate(chunks)]
                ps2s = [ps.tile([P, 512], F32, tag="ps2", name=f"ps2_{i}")[:, :tl]
                        for i, (t0, tl) in enumerate(chunks)]
                pre1 = psb.tile([P, 512], F32, tag="psy", name="pre1")[:, :tl0]
                pre2 = psb.tile([P, 512], F32, tag="psy", name="pre2")[:, :tl0]
                for ho in range(HO):
                    st, sp_ = (ho == 0), (ho == HO - 1)
                    for i, (t0, tl) in enumerate(chunks):
                        nc.tensor.matmul(ps1s[i], wsl(wv0, 0, ho),
                                         xgs[ho][:, t0:t0 + tl], start=st, stop=sp_)
                        nc.tensor.matmul(ps2s[i], wsl(wv0, 1, ho),
                                         xgs[ho][:, t0:t0 + tl], start=st, stop=sp_)
                    nc.tensor.matmul(pre1, wsl(wv1, 0, ho),
                                     xgs[ho][:, t00:t00 + tl0], start=st, stop=sp_)
                    nc.tensor.matmul(pre2, wsl(wv1, 1, ho),
                                     xgs[ho][:, t00:t00 + tl0], start=st, stop=sp_)
                for i, (t0, tl) in enumerate(chunks):
                    glu_tail(0, t0, tl, ps1s[i], ps2s[i])
                glu_tail(1, t00, tl0, pre1, pre2)

                for fo in range(1, FO):
                    if fo == 1:
                        wv = wv1
                        fo_chunks = chunks[1:]
                    else:
                        wv = load_wv(fo)
                        fo_chunks = chunks
                    ps1s = [ps.tile([P, 512], F32, tag="ps1", name=f"ps1_{i}")[:, :tl]
                            for i, (t0, tl) in enumerate(fo_chunks)]
                    ps2s = [ps.tile([P, 512], F32, tag="ps2", name=f"ps2_{i}")[:, :tl]
                            for i, (t0, tl) in enumerate(fo_chunks)]
                    for ho in range(HO):
                        st, sp_ = (ho == 0), (ho == HO - 1)
                        for i, (t0, tl) in enumerate(fo_chunks):
                            nc.tensor.matmul(ps1s[i], wsl(wv, 0, ho),
                                             xgs[ho][:, t0:t0 + tl],
                                             start=st, stop=sp_)
                            nc.tensor.matmul(ps2s[i], wsl(wv, 1, ho),
                                             xgs[ho][:, t0:t0 + tl],
                                             start=st, stop=sp_)
                    for i, (t0, tl) in enumerate(fo_chunks):
                        glu_tail(fo, t0, tl, ps1s[i], ps2s[i])

            # Phase B: y[t, :] = (h^T @ w2) * c[t]
            for hh in range(2):
                w2tiles = []
                for fo in range(FO):
                    w2s = w2p.tile([P, H // 2], F32R, tag="w2s",
                                   name=f"w2s_{hh}_{fo}")
                    nc.sync.dma_start(w2s[:], w2t.ap()[hh, :, fo, :])
                    w2tiles.append(w2s)
                for to in range(TO):
                    last = (hh == 1 and to == TO - 1)
                    # Final block: two half-width groups so the first half's
                    # gate-mul + store hide under the second half's matmuls.
                    parts = ((0, 256), (256, 256)) if last else ((0, 512),)
                    for (h0, hl) in parts:
                        psy = psb.tile([P, 512], F32, tag="psy",
                                       name="psy")[:, :hl]
                        for fo in range(FO):
                            nc.tensor.matmul(psy,
                                             h[:, fo, to * P:(to + 1) * P],
                                             w2tiles[fo][:, h0:h0 + hl],
                                             start=(fo == 0),
                                             stop=(fo == FO - 1))
                        yt = yp.tile([P, 512], F32, tag="yt", name="yt")[:, :hl]
                        nc.vector.tensor_scalar_mul(yt, psy, cgs[:, to:to + 1])
                        nc.sync.dma_start(
                            y.ap()[to * P:(to + 1) * P,
                                   hh * 512 + h0:hh * 512 + h0 + hl], yt)
    nc.compile()
    return nc


def _get_nc(key, builder):
    if key not in _NC_CACHE:
        _NC_CACHE[key] = builder()
    return _NC_CACHE[key]


def _tile_weights(w1, v1, w2):
    """Pre-tile the expert weights for large-descriptor DMA.

    w1t/v1t: [E, FO, 128(h), HO, 128(f)]  (lhsT tiles of [H,F] transposed mats)
    w2t:     [E, 2, 128(f), FO, 512(h)]
    """
    key = (w1.shape, w1.dtype.str, w1[0, 0, :4].tobytes(), w2[0, 0, :4].tobytes(),
           v1[0, 0, :4].tobytes(), float(w1[-1, -1, -1]), float(w2[-1, -1, -1]))
    if key in _W_CACHE:
        return _W_CACHE[key]
    # w1[e] is [F, H]; lhsT tile (fo): [p_h, ho, q_f] = w1[e][fo*128+q, ho*128+p]
    w1t = np.ascontiguousarray(
        w1.reshape(E, FO, P, HO, P).transpose(0, 1, 4, 3, 2))
    v1t = np.ascontiguousarray(
        v1.reshape(E, FO, P, HO, P).transpose(0, 1, 4, 3, 2))
    # w2[e] is [F, H]; tile (hh): [p_f, fo, j_h] = w2[e][fo*128+p, hh*512+j]
    w2t = np.ascontiguousarray(
        w2.reshape(E, FO, P, 2, H // 2).transpose(0, 3, 2, 1, 4))
    _W_CACHE.clear()
    _W_CACHE[key] = (w1t, v1t, w2t)
    return w1t, v1t, w2t


def kernel(x, router_w, w1, v1, w2):
    x = np.asarray(x, dtype=np.float32)
    router_w = np.asarray(router_w, dtype=np.float32)
    w1 = np.asarray(w1, dtype=np.float32)
    v1 = np.asarray(v1, dtype=np.float32)
    w2 = np.asarray(w2, dtype=np.float32)

    xf = x.reshape(T, H)
    xT = np.ascontiguousarray(xf.T)  # [H, T]
    rwT = np.ascontiguousarray(router_w.T)  # [H, E]

    # ---- Phase 1: router on device (data-parallel over tokens) ----
    nc1 = _get_nc("router", _build_router)
    in1 = [{"xT": np.ascontiguousarray(xT[:, i * TSH:(i + 1) * TSH]), "rwT": rwT}
           for i in range(NCORES)]
    r1 = run_bass_kernel_spmd(nc1, in1, core_ids=list(range(NCORES)))
    c = np.concatenate([r["c"] for r in r1.results], axis=0)  # [T, E]

    # ---- Host dispatch: gather tokens per expert (data movement only) ----
    idxs = [np.flatnonzero(c[:, e] != 0.0) for e in range(E)]
    maxc = max(len(ix) for ix in idxs)
    # Per-launch capacity; >1280 tokens per expert (never happens with
    # balanced routing) is handled by running the same NEFF multiple times.
    C = max(1152, min(1280, ((maxc + 127) // 128) * 128))
    nseg = (maxc + C - 1) // C

    w1t, v1t, w2t = _tile_weights(w1, v1, w2)

    out = np.zeros((T, H), np.float32)
    for seg in range(nseg):
        segixs = [idxs[e][seg * C:(seg + 1) * C] for e in range(E)]
        CA = max(1, max(len(ix) for ix in segixs))  # exact active count
        TO = (CA + P - 1) // P
        nc2 = _get_nc(("expert", C, CA), lambda: _build_expert(C, CA))
        in2 = []
        for e in range(E):
            ix = segixs[e]
            xgT = np.zeros((H, C), np.float32)
            xgT[:, :len(ix)] = xT[:, ix]
            cge = np.zeros((TO * P,), np.float32)
            cge[:len(ix)] = c[ix, e]
            cgt = np.ascontiguousarray(cge.reshape(TO, P).T)  # [P, TO]
            in2.append({"xgT": xgT, "cgt": cgt,
                        "w1t": w1t[e], "v1t": v1t[e], "w2t": w2t[e]})
        r2 = run_bass_kernel_spmd(nc2, in2, core_ids=list(range(NCORES)))
        # ---- Host combine: scatter-add per-expert outputs ----
        for e in range(E):
            ix = segixs[e]
            out[ix] += r2.results[e]["y"][:len(ix)]
    return out.reshape(x.shape)

